# revision 7
# baseline (speedup 1.0000x reference)
"""Trainium2 Bass kernel for nn_DependencyParsing (embedding_lookup).

Strategy (pure data-parallel over 8 NeuronCores, B=65536 -> 8192/core):
  - Tables are cast to bf16 and padded to 256B rows; word rows are gathered
    feature-major straight into SBUF via the SWDGE transpose dma_gather
    (max 512 indices per instruction on HW).
  - pos/dep lookups are fused into a single [50*45, 256B] table whose rows
    are [pos_row(100), 1.0@col100, pad, dep_row(100), pad]; the constant 1.0
    column carries the combined bias (bw+bp+bd) via an extra weight row.
  - h = x @ W runs as bf16 matmuls (21 K-blocks x 6 M-tiles per 512-batch
    chunk) accumulating f32 in PSUM; h^3 = Square(h) * h on ACT+DVE,
    output cast to bf16.
  - logits = h3 @ Wo (bias bo folded via a constant-1 row), then PE-mode
    transpose to batch-major and a max-free softmax (logits are tiny, so
    exp is safe without the max subtraction) on ACT/DVE.
"""

import numpy as np
import ml_dtypes

import concourse.bacc as bacc
import concourse.mybir as mybir
import concourse.tile as tile
from concourse.masks import make_identity
from concourse.bass_utils import run_bass_kernel_spmd

B, T, D, H, V, NPOS, NDEP, OUT = 65536, 7, 100, 700, 32000, 50, 45, 93
NCORES = 8
B_CORE = B // NCORES
CHUNK = 512
PD_ROWS = NPOS * NDEP  # 2250
P = 128
# M-tiles over the 700 output features of h
MT = [(0, 128), (128, 128), (256, 128), (384, 128), (512, 128), (640, 60)]
# K-blocks for logits: 700 h-features in 6 blocks of 128 (last 60)
LKB = [(0, 128), (128, 128), (256, 128), (384, 128), (512, 128), (640, 60)]
dt = mybir.dt
bf16 = ml_dtypes.bfloat16

_NC_CACHE = {}


def build_nc(b_core):
    n_chunks = b_core // CHUNK
    nc = bacc.Bacc(None, target_bir_lowering=False)
    with tile.TileContext(nc) as tc:
        with tc.tile_pool(name="dram", bufs=1, space="DRAM") as dram:
            word_tab = dram.tile([V + 1, 128], dt.bfloat16, kind="ExternalInput",
                                 name="word_tab", uniquify=False)
            pd_tab = dram.tile([PD_ROWS, 256], dt.bfloat16, kind="ExternalInput",
                               name="pd_tab", uniquify=False)
            widx_d = dram.tile([P, T * n_chunks * 32], dt.int16, kind="ExternalInput",
                               name="widx", uniquify=False)
            pdidx_d = dram.tile([P, T * n_chunks * 32], dt.int16, kind="ExternalInput",
                                name="pdidx", uniquify=False)
            ww_d = dram.tile([P, T * H], dt.bfloat16, kind="ExternalInput",
                             name="w_word", uniquify=False)
            wpd_d = dram.tile([P, 2 * T * H], dt.bfloat16, kind="ExternalInput",
                              name="w_pd", uniquify=False)
            wo_d = dram.tile([P, 6 * 96], dt.bfloat16, kind="ExternalInput",
                             name="w_o", uniquify=False)
            bo_d = dram.tile([P, 1], dt.float32, kind="ExternalInput",
                             name="bo_pad", uniquify=False)
            out_d = dram.tile([n_chunks * 4, P, OUT], dt.float32, kind="ExternalOutput",
                              name="out", uniquify=False)

            with (
                tc.tile_pool(name="const", bufs=1) as const,
                tc.tile_pool(name="wg", bufs=2) as wg_pool,
                tc.tile_pool(name="pdg", bufs=2) as pdg_pool,
                tc.tile_pool(name="sq", bufs=2) as sq_pool,
                tc.tile_pool(name="h3", bufs=2) as h3_pool,
                tc.tile_pool(name="lg", bufs=2) as lg_pool,
                tc.tile_pool(name="ex", bufs=2) as ex_pool,
                tc.tile_pool(name="sm", bufs=2) as sm_pool,
                tc.tile_pool(name="op", bufs=2) as op_pool,
                tc.tile_pool(name="hps", bufs=1, space="PSUM") as hps_pool,
                tc.tile_pool(name="ltps", bufs=2, space="PSUM") as ltps_pool,
            ):
                ww_sb = const.tile([P, T * H], dt.bfloat16, name="ww_sb")
                nc.sync.dma_start(out=ww_sb[:], in_=ww_d[:])
                wpd_sb = const.tile([P, 2 * T * H], dt.bfloat16, name="wpd_sb")
                nc.sync.dma_start(out=wpd_sb[:], in_=wpd_d[:])
                wo_sb = const.tile([P, 6 * 96], dt.bfloat16, name="wo_sb")
                nc.sync.dma_start(out=wo_sb[:], in_=wo_d[:])
                widx_sb = const.tile([P, T * n_chunks * 32], dt.int16, name="widx_sb")
                nc.sync.dma_start(out=widx_sb[:], in_=widx_d[:])
                pdidx_sb = const.tile([P, T * n_chunks * 32], dt.int16, name="pdidx_sb")
                nc.sync.dma_start(out=pdidx_sb[:], in_=pdidx_d[:])
                bo_sb = const.tile([P, 1], dt.float32, name="bo_sb")
                nc.sync.dma_start(out=bo_sb[:], in_=bo_d[:])
                ident = const.tile([P, P], dt.float32, name="ident")
                make_identity(nc, ident)

                for c in range(n_chunks):
                    # ---- gathers (feature-major) ----
                    wg = []
                    pdg = []
                    for t in range(T):
                        g = wg_pool.tile([P, CHUNK], dt.bfloat16, name=f"wg{t}")
                        nc.gpsimd.dma_gather(
                            g.rearrange("p (o n) -> p o n", o=1),
                            word_tab[:],
                            widx_sb[:, (t * n_chunks + c) * 32:(t * n_chunks + c + 1) * 32],
                            CHUNK, CHUNK, 128, transpose=True,
                        )
                        wg.append(g)
                    for t in range(T):
                        g = pdg_pool.tile([P, 2 * CHUNK], dt.bfloat16, name=f"pdg{t}")
                        nc.gpsimd.dma_gather(
                            g.rearrange("p (o n) -> p o n", o=2),
                            pd_tab[:],
                            pdidx_sb[:, (t * n_chunks + c) * 32:(t * n_chunks + c + 1) * 32],
                            CHUNK, CHUNK, 256, transpose=True,
                        )
                        pdg.append(g)

                    # ---- h = x @ W (+bias via 1-col), h3 = (h)^2 * h ----
                    h3 = []
                    for mi, (m0, msz) in enumerate(MT):
                        hp = hps_pool.tile([P, CHUNK], dt.float32, name=f"hps{mi}")
                        kb = 0
                        for t in range(T):
                            nc.tensor.matmul(
                                hp[:msz, :],
                                ww_sb[:, t * H + m0: t * H + m0 + msz],
                                wg[t][:, :],
                                start=(kb == 0), stop=(kb == 20),
                            )
                            kb += 1
                        for t in range(T):
                            for half in range(2):
                                nc.tensor.matmul(
                                    hp[:msz, :],
                                    wpd_sb[:, (t * 2 + half) * H + m0:
                                           (t * 2 + half) * H + m0 + msz],
                                    g3 := pdg[t].rearrange("p (o n) -> p o n", o=2)[:, half, :],
                                    start=(kb == 0), stop=(kb == 20),
                                )
                                kb += 1
                        sq = sq_pool.tile([P, CHUNK], dt.float32, name="sq")
                        nc.scalar.square(sq[:msz, :], hp[:msz, :])
                        h3t = h3_pool.tile([P, CHUNK], dt.bfloat16, name=f"h3_{mi}")
                        nc.vector.tensor_mul(h3t[:msz, :], sq[:msz, :], hp[:msz, :])
                        h3.append(h3t)

                    # ---- logits = h3 @ Wo (+bo via 1-row) ----
                    lg_ps = ltps_pool.tile([P, CHUNK], dt.float32, name="lg_ps", tag="lt")
                    for j, (k0, ksz) in enumerate(LKB):
                        nc.tensor.matmul(
                            lg_ps[:96, :],
                            wo_sb[:ksz, j * 96:(j + 1) * 96],
                            h3[j][:ksz, :],
                            start=(j == 0), stop=(j == 5),
                        )
                    lg_sb = lg_pool.tile([P, CHUNK], dt.float32, name="lg_sb")
                    nc.scalar.activation(lg_sb[:96, :], lg_ps[:96, :],
                                         mybir.ActivationFunctionType.Identity,
                                         bias=bo_sb[:96, :])

                    # ---- transpose to batch-major + softmax (no max sub) ----
                    for q in range(4):
                        tp = ltps_pool.tile([P, 96], dt.float32, name="tp", tag="lt")
                        nc.tensor.transpose(
                            tp[:, :96],
                            lg_sb[:96, q * 128:(q + 1) * 128],
                            ident[:96, :96],
                        )
                        ex = ex_pool.tile([P, 96], dt.float32, name="ex")
                        nc.scalar.activation(ex[:, :], tp[:, :],
                                             mybir.ActivationFunctionType.Exp)
                        sm = sm_pool.tile([P, 1], dt.float32, name="sm")
                        nc.vector.tensor_reduce(sm[:, :], ex[:, :OUT],
                                                axis=mybir.AxisListType.X,
                                                op=mybir.AluOpType.add)
                        rc = sm_pool.tile([P, 1], dt.float32, name="rc")
                        nc.vector.reciprocal(rc[:, :], sm[:, :])
                        opt = op_pool.tile([P, OUT], dt.float32, name="opt")
                        nc.vector.tensor_scalar_mul(opt[:, :], ex[:, :OUT], rc[:, :])
                        nc.sync.dma_start(out=out_d[c * 4 + q], in_=opt[:, :])
    nc.compile()
    return nc


def _wrap_idx(idx_tc):
    """[n_chunks*CHUNK] per (t) -> [128, n_chunks*32] wrapped+replicated."""
    n = idx_tc.shape[0]
    w = idx_tc.reshape(n // 16, 16).T  # [16, n/16]
    return np.tile(w, (8, 1))


def prep_inputs(word_idx, pos_idx, dep_idx, word_table, pos_table, dep_table,
                Ww, bw, Wp, bp, Wd, bd, Wo, bo, b_core):
    """Returns (shared_map, per_core_fn). Host work is layout-only + small."""
    n_chunks = b_core // CHUNK

    wt = np.zeros((V + 1, 128), dtype=bf16)
    wt[:V, :D] = np.asarray(word_table, np.float32).astype(bf16)

    pdt = np.zeros((PD_ROWS, 256), dtype=bf16)
    pt = np.asarray(pos_table, np.float32).astype(bf16)
    dtab = np.asarray(dep_table, np.float32).astype(bf16)
    pdt[:, :D] = np.repeat(pt, NDEP, axis=0)
    pdt[:, 100] = np.float32(1.0)
    pdt[:, 128:128 + D] = np.tile(dtab, (NPOS, 1))

    bias_all = (np.asarray(bw, np.float32) + np.asarray(bp, np.float32)
                + np.asarray(bd, np.float32))

    def pack_w(Wmat, bias_row=None):
        # [T, 128, H] with rows 0..99 = W[100t:100t+100], optional bias row.
        arr = np.zeros((T, P, H), dtype=bf16)
        Wmat = np.asarray(Wmat, np.float32)
        for t in range(T):
            arr[t, :D, :] = Wmat[D * t:D * (t + 1), :].astype(bf16)
        if bias_row is not None:
            arr[0, 100, :] = bias_row.astype(bf16)
        return arr

    ww = pack_w(Ww)                                # [7,128,700]
    wp = pack_w(Wp, bias_all)
    wd = pack_w(Wd)
    wpd = np.zeros((T, 2, P, H), dtype=bf16)
    wpd[:, 0] = wp
    wpd[:, 1] = wd

    wo = np.zeros((6, P, 96), dtype=bf16)
    Wo32 = np.asarray(Wo, np.float32)
    for j, (k0, ksz) in enumerate(LKB):
        wo[j, :ksz, :OUT] = Wo32[k0:k0 + ksz, :].astype(bf16)

    bo_pad = np.zeros((P, 1), dtype=np.float32)
    bo_pad[:OUT, 0] = np.asarray(bo, np.float32)

    shared = {
        "word_tab": wt,
        "pd_tab": pdt,
        "w_word": np.ascontiguousarray(ww.transpose(1, 0, 2)).reshape(P, T * H),
        "w_pd": np.ascontiguousarray(wpd.transpose(2, 0, 1, 3)).reshape(P, 2 * T * H),
        "w_o": np.ascontiguousarray(wo.transpose(1, 0, 2)).reshape(P, 6 * 96),
        "bo_pad": bo_pad,
    }

    wi = np.asarray(word_idx, np.int64).copy()
    wi[wi < 0] = V
    wi = wi.astype(np.int16)
    fidx = (np.asarray(pos_idx, np.int64) * NDEP
            + np.asarray(dep_idx, np.int64)).astype(np.int16)

    def core_map(core):
        s = slice(core * b_core, (core + 1) * b_core)
        wic, fic = wi[s], fidx[s]
        widx = np.zeros((P, T, n_chunks, 32), dtype=np.int16)
        pdidx = np.zeros((P, T, n_chunks, 32), dtype=np.int16)
        for t in range(T):
            for c in range(n_chunks):
                widx[:, t, c, :] = _wrap_idx(wic[c * CHUNK:(c + 1) * CHUNK, t])
                pdidx[:, t, c, :] = _wrap_idx(fic[c * CHUNK:(c + 1) * CHUNK, t])
        m = dict(shared)
        m["widx"] = widx.reshape(P, T * n_chunks * 32)
        m["pdidx"] = pdidx.reshape(P, T * n_chunks * 32)
        return m

    return shared, core_map


def kernel(**inputs):
    b_core = B_CORE
    if b_core not in _NC_CACHE:
        _NC_CACHE[b_core] = build_nc(b_core)
    nc = _NC_CACHE[b_core]

    _, core_map = prep_inputs(b_core=b_core, **inputs)
    in_maps = [core_map(i) for i in range(NCORES)]
    res = run_bass_kernel_spmd(nc, in_maps, core_ids=list(range(NCORES)))
    outs = [r["out"].reshape(b_core, OUT) for r in res.results]
    return np.concatenate(outs, axis=0).astype(np.float32)


# revision 9
# speedup vs baseline: 2.1589x; 2.1589x over previous
"""Trainium2 Bass kernel for nn_DependencyParsing (embedding_lookup).

Strategy (pure data-parallel over 8 NeuronCores, B=65536 -> 8192/core):
  - Tables are cast to bf16 and padded to 256B rows; word rows are gathered
    feature-major straight into SBUF via the SWDGE transpose dma_gather
    (max 512 indices per instruction on HW).
  - pos/dep lookups are fused into a single [50*45, 256B] table whose rows
    are [pos_row(100), 1.0@col100, pad, dep_row(100), pad]; the constant 1.0
    column carries the combined bias (bw+bp+bd) via an extra weight row.
  - h = x @ W runs as bf16 matmuls (21 K-blocks x 6 M-tiles per 512-batch
    chunk) accumulating f32 in PSUM; h^3 = Square(h) * h on ACT+DVE,
    output cast to bf16.
  - logits = h3 @ Wo (bias bo folded via a constant-1 row), then PE-mode
    transpose to batch-major and a max-free softmax (logits are tiny, so
    exp is safe without the max subtraction) on ACT/DVE.
"""

import numpy as np
import ml_dtypes

import concourse.bacc as bacc
import concourse.mybir as mybir
import concourse.tile as tile
from concourse.masks import make_identity
from concourse.bass_utils import run_bass_kernel_spmd

B, T, D, H, V, NPOS, NDEP, OUT = 65536, 7, 100, 700, 32000, 50, 45, 93
NCORES = 8
B_CORE = B // NCORES
CHUNK = 512
PD_ROWS = NPOS * NDEP  # 2250
P = 128
# M-tiles over the 700 output features of h
MT = [(0, 128), (128, 128), (256, 128), (384, 128), (512, 128), (640, 60)]
# K-blocks for logits: 700 h-features in 6 blocks of 128 (last 60)
LKB = [(0, 128), (128, 128), (256, 128), (384, 128), (512, 128), (640, 60)]
dt = mybir.dt
bf16 = ml_dtypes.bfloat16

_NC_CACHE = {}


def build_nc(b_core):
    n_chunks = b_core // CHUNK
    nc = bacc.Bacc(None, target_bir_lowering=False, num_swdge_queues=4)
    with tile.TileContext(nc) as tc:
        with tc.tile_pool(name="dram", bufs=1, space="DRAM") as dram:
            word_tab = dram.tile([V + 1, 128], dt.bfloat16, kind="ExternalInput",
                                 name="word_tab", uniquify=False)
            pd_tab = dram.tile([PD_ROWS, 256], dt.bfloat16, kind="ExternalInput",
                               name="pd_tab", uniquify=False)
            widx_d = dram.tile([P, T * n_chunks * 32], dt.int16, kind="ExternalInput",
                               name="widx", uniquify=False)
            pdidx_d = dram.tile([P, T * n_chunks * 32], dt.int16, kind="ExternalInput",
                                name="pdidx", uniquify=False)
            ww_d = dram.tile([P, T * H], dt.bfloat16, kind="ExternalInput",
                             name="w_word", uniquify=False)
            wpd_d = dram.tile([P, 2 * T * H], dt.bfloat16, kind="ExternalInput",
                              name="w_pd", uniquify=False)
            wo_d = dram.tile([P, 6 * 96], dt.bfloat16, kind="ExternalInput",
                             name="w_o", uniquify=False)
            bo_d = dram.tile([P, 1], dt.float32, kind="ExternalInput",
                             name="bo_pad", uniquify=False)
            out_d = dram.tile([n_chunks * 4, P, OUT], dt.float32, kind="ExternalOutput",
                              name="out", uniquify=False)

            with (
                tc.tile_pool(name="const", bufs=1) as const,
                tc.tile_pool(name="wg", bufs=2) as wg_pool,
                tc.tile_pool(name="pdg", bufs=2) as pdg_pool,
                tc.tile_pool(name="sq", bufs=2) as sq_pool,
                tc.tile_pool(name="h3", bufs=2) as h3_pool,
                tc.tile_pool(name="lg", bufs=2) as lg_pool,
                tc.tile_pool(name="ex", bufs=2) as ex_pool,
                tc.tile_pool(name="sm", bufs=2) as sm_pool,
                tc.tile_pool(name="op", bufs=2) as op_pool,
                tc.tile_pool(name="hps", bufs=1, space="PSUM") as hps_pool,
                tc.tile_pool(name="ltps", bufs=2, space="PSUM") as ltps_pool,
            ):
                ww_sb = const.tile([P, T * H], dt.bfloat16, name="ww_sb")
                nc.sync.dma_start(out=ww_sb[:], in_=ww_d[:])
                wpd_sb = const.tile([P, 2 * T * H], dt.bfloat16, name="wpd_sb")
                nc.sync.dma_start(out=wpd_sb[:], in_=wpd_d[:])
                wo_sb = const.tile([P, 6 * 96], dt.bfloat16, name="wo_sb")
                nc.sync.dma_start(out=wo_sb[:], in_=wo_d[:])
                widx_sb = const.tile([P, T * n_chunks * 32], dt.int16, name="widx_sb")
                nc.sync.dma_start(out=widx_sb[:], in_=widx_d[:])
                pdidx_sb = const.tile([P, T * n_chunks * 32], dt.int16, name="pdidx_sb")
                nc.sync.dma_start(out=pdidx_sb[:], in_=pdidx_d[:])
                bo_sb = const.tile([P, 1], dt.float32, name="bo_sb")
                nc.sync.dma_start(out=bo_sb[:], in_=bo_d[:])
                ident = const.tile([P, P], dt.float32, name="ident")
                make_identity(nc, ident)

                qn = 0
                for c in range(n_chunks):
                    # ---- gathers (feature-major), cycled over 4 SWDGE queues ----
                    wg = []
                    pdg = []
                    for t in range(T):
                        g = wg_pool.tile([P, CHUNK], dt.bfloat16, name=f"wg{t}")
                        nc.gpsimd.dma_gather(
                            g.rearrange("p (o n) -> p o n", o=1),
                            word_tab[:],
                            widx_sb[:, (t * n_chunks + c) * 32:(t * n_chunks + c + 1) * 32],
                            CHUNK, CHUNK, 128, transpose=True, queue_num=qn % 4,
                        )
                        qn += 1
                        wg.append(g)
                    for t in range(T):
                        g = pdg_pool.tile([P, 2 * CHUNK], dt.bfloat16, name=f"pdg{t}")
                        nc.gpsimd.dma_gather(
                            g.rearrange("p (o n) -> p o n", o=2),
                            pd_tab[:],
                            pdidx_sb[:, (t * n_chunks + c) * 32:(t * n_chunks + c + 1) * 32],
                            CHUNK, CHUNK, 256, transpose=True, queue_num=qn % 4,
                        )
                        qn += 1
                        pdg.append(g)

                    # ---- h = x @ W (+bias via 1-col), h3 = (h)^2 * h ----
                    h3 = []
                    for mi, (m0, msz) in enumerate(MT):
                        hp = hps_pool.tile([P, CHUNK], dt.float32, name=f"hps{mi}")
                        kb = 0
                        for t in range(T):
                            nc.tensor.matmul(
                                hp[:msz, :],
                                ww_sb[:, t * H + m0: t * H + m0 + msz],
                                wg[t][:, :],
                                start=(kb == 0), stop=(kb == 20),
                            )
                            kb += 1
                        for t in range(T):
                            for half in range(2):
                                nc.tensor.matmul(
                                    hp[:msz, :],
                                    wpd_sb[:, (t * 2 + half) * H + m0:
                                           (t * 2 + half) * H + m0 + msz],
                                    g3 := pdg[t].rearrange("p (o n) -> p o n", o=2)[:, half, :],
                                    start=(kb == 0), stop=(kb == 20),
                                )
                                kb += 1
                        sq = sq_pool.tile([P, CHUNK], dt.float32, name="sq")
                        nc.scalar.square(sq[:msz, :], hp[:msz, :])
                        h3t = h3_pool.tile([P, CHUNK], dt.bfloat16, name=f"h3_{mi}")
                        nc.vector.tensor_mul(h3t[:msz, :], sq[:msz, :], hp[:msz, :])
                        h3.append(h3t)

                    # ---- logits = h3 @ Wo (+bo via 1-row) ----
                    lg_ps = ltps_pool.tile([P, CHUNK], dt.float32, name="lg_ps", tag="lt")
                    for j, (k0, ksz) in enumerate(LKB):
                        nc.tensor.matmul(
                            lg_ps[:96, :],
                            wo_sb[:ksz, j * 96:(j + 1) * 96],
                            h3[j][:ksz, :],
                            start=(j == 0), stop=(j == 5),
                        )
                    lg_sb = lg_pool.tile([P, CHUNK], dt.float32, name="lg_sb")
                    nc.scalar.activation(lg_sb[:96, :], lg_ps[:96, :],
                                         mybir.ActivationFunctionType.Identity,
                                         bias=bo_sb[:96, :])

                    # ---- transpose to batch-major + softmax (no max sub) ----
                    for q in range(4):
                        tp = ltps_pool.tile([P, 96], dt.float32, name="tp", tag="lt")
                        nc.tensor.transpose(
                            tp[:, :96],
                            lg_sb[:96, q * 128:(q + 1) * 128],
                            ident[:96, :96],
                        )
                        ex = ex_pool.tile([P, 96], dt.float32, name="ex")
                        nc.scalar.activation(ex[:, :], tp[:, :],
                                             mybir.ActivationFunctionType.Exp)
                        sm = sm_pool.tile([P, 1], dt.float32, name="sm")
                        nc.vector.tensor_reduce(sm[:, :], ex[:, :OUT],
                                                axis=mybir.AxisListType.X,
                                                op=mybir.AluOpType.add)
                        rc = sm_pool.tile([P, 1], dt.float32, name="rc")
                        nc.vector.reciprocal(rc[:, :], sm[:, :])
                        opt = op_pool.tile([P, OUT], dt.float32, name="opt")
                        nc.vector.tensor_scalar_mul(opt[:, :], ex[:, :OUT], rc[:, :])
                        nc.sync.dma_start(out=out_d[c * 4 + q], in_=opt[:, :])
    nc.compile()
    return nc


def _wrap_idx(idx_tc):
    """[n_chunks*CHUNK] per (t) -> [128, n_chunks*32] wrapped+replicated."""
    n = idx_tc.shape[0]
    w = idx_tc.reshape(n // 16, 16).T  # [16, n/16]
    return np.tile(w, (8, 1))


def prep_inputs(word_idx, pos_idx, dep_idx, word_table, pos_table, dep_table,
                Ww, bw, Wp, bp, Wd, bd, Wo, bo, b_core):
    """Returns (shared_map, per_core_fn). Host work is layout-only + small."""
    n_chunks = b_core // CHUNK

    wt = np.zeros((V + 1, 128), dtype=bf16)
    wt[:V, :D] = np.asarray(word_table, np.float32).astype(bf16)

    pdt = np.zeros((PD_ROWS, 256), dtype=bf16)
    pt = np.asarray(pos_table, np.float32).astype(bf16)
    dtab = np.asarray(dep_table, np.float32).astype(bf16)
    pdt[:, :D] = np.repeat(pt, NDEP, axis=0)
    pdt[:, 100] = np.float32(1.0)
    pdt[:, 128:128 + D] = np.tile(dtab, (NPOS, 1))

    bias_all = (np.asarray(bw, np.float32) + np.asarray(bp, np.float32)
                + np.asarray(bd, np.float32))

    def pack_w(Wmat, bias_row=None):
        # [T, 128, H] with rows 0..99 = W[100t:100t+100], optional bias row.
        arr = np.zeros((T, P, H), dtype=bf16)
        Wmat = np.asarray(Wmat, np.float32)
        for t in range(T):
            arr[t, :D, :] = Wmat[D * t:D * (t + 1), :].astype(bf16)
        if bias_row is not None:
            arr[0, 100, :] = bias_row.astype(bf16)
        return arr

    ww = pack_w(Ww)                                # [7,128,700]
    wp = pack_w(Wp, bias_all)
    wd = pack_w(Wd)
    wpd = np.zeros((T, 2, P, H), dtype=bf16)
    wpd[:, 0] = wp
    wpd[:, 1] = wd

    wo = np.zeros((6, P, 96), dtype=bf16)
    Wo32 = np.asarray(Wo, np.float32)
    for j, (k0, ksz) in enumerate(LKB):
        wo[j, :ksz, :OUT] = Wo32[k0:k0 + ksz, :].astype(bf16)

    bo_pad = np.zeros((P, 1), dtype=np.float32)
    bo_pad[:OUT, 0] = np.asarray(bo, np.float32)

    shared = {
        "word_tab": wt,
        "pd_tab": pdt,
        "w_word": np.ascontiguousarray(ww.transpose(1, 0, 2)).reshape(P, T * H),
        "w_pd": np.ascontiguousarray(wpd.transpose(2, 0, 1, 3)).reshape(P, 2 * T * H),
        "w_o": np.ascontiguousarray(wo.transpose(1, 0, 2)).reshape(P, 6 * 96),
        "bo_pad": bo_pad,
    }

    wi = np.asarray(word_idx, np.int64).copy()
    wi[wi < 0] = V
    wi = wi.astype(np.int16)
    fidx = (np.asarray(pos_idx, np.int64) * NDEP
            + np.asarray(dep_idx, np.int64)).astype(np.int16)

    def core_map(core):
        s = slice(core * b_core, (core + 1) * b_core)
        wic, fic = wi[s], fidx[s]
        widx = np.zeros((P, T, n_chunks, 32), dtype=np.int16)
        pdidx = np.zeros((P, T, n_chunks, 32), dtype=np.int16)
        for t in range(T):
            for c in range(n_chunks):
                widx[:, t, c, :] = _wrap_idx(wic[c * CHUNK:(c + 1) * CHUNK, t])
                pdidx[:, t, c, :] = _wrap_idx(fic[c * CHUNK:(c + 1) * CHUNK, t])
        m = dict(shared)
        m["widx"] = widx.reshape(P, T * n_chunks * 32)
        m["pdidx"] = pdidx.reshape(P, T * n_chunks * 32)
        return m

    return shared, core_map


def kernel(**inputs):
    b_core = B_CORE
    if b_core not in _NC_CACHE:
        _NC_CACHE[b_core] = build_nc(b_core)
    nc = _NC_CACHE[b_core]

    _, core_map = prep_inputs(b_core=b_core, **inputs)
    in_maps = [core_map(i) for i in range(NCORES)]
    res = run_bass_kernel_spmd(nc, in_maps, core_ids=list(range(NCORES)))
    outs = [r["out"].reshape(b_core, OUT) for r in res.results]
    return np.concatenate(outs, axis=0).astype(np.float32)


# revision 11
# speedup vs baseline: 2.1640x; 1.0024x over previous
"""Trainium2 Bass kernel for nn_DependencyParsing (embedding_lookup).

Strategy (pure data-parallel over 8 NeuronCores, B=65536 -> 8192/core):
  - word_table cast to bf16, rows padded to 256B; word embeddings gathered
    feature-major straight into SBUF via SWDGE transpose dma_gather
    (512 idx / instruction HW cap), cycled over 4 SWDGE queues (the
    single-queue descriptor ring serializes at ~4.75us/gather otherwise).
  - pos/dep lookups use no gather at all: pe@Wp + de@Wd is computed as
    onehot @ proj, where proj[s*64+cls] = table_s[cls] @ W_s (built on
    device, 28 small matmuls) and the one-hot [128, 512] per (t) comes
    from a single DVE is_equal of host-replicated fp16 index rows against
    a per-partition iota. Slot s=pos_t on partitions 0..63, dep_t on
    64..127. The combined bias (bw+bp+bd) rides a constant-1 row (t=0,
    partition 63) with proj row 63 = bias.
  - h = x @ W as bf16 matmuls (14 K-blocks x 6 M-tiles per 512 chunk)
    accumulating f32 in PSUM; h^3 = Square(h)*h on ACT+DVE -> bf16.
  - logits = h3 @ Wo (bo added via ACT Identity-with-bias on the PSUM
    eviction), PE-mode transpose to batch-major, then a max-free softmax
    (logits are tiny so exp is safe) on ACT/DVE.
"""

import numpy as np
import ml_dtypes

import concourse.bacc as bacc
import concourse.mybir as mybir
import concourse.tile as tile
from concourse.masks import make_identity
from concourse.bass_utils import run_bass_kernel_spmd

B, T, D, H, V, NPOS, NDEP, OUT = 65536, 7, 100, 700, 32000, 50, 45, 93
NCORES = 8
B_CORE = B // NCORES
CHUNK = 512
P = 128
# M-tiles over the 700 output features of h
MT = [(0, 128), (128, 128), (256, 128), (384, 128), (512, 128), (640, 60)]
# K-blocks for logits: 700 h-features in 6 blocks of 128 (last 60)
LKB = [(0, 128), (128, 128), (256, 128), (384, 128), (512, 128), (640, 60)]
dt = mybir.dt
bf16 = ml_dtypes.bfloat16

_NC_CACHE = {}


def build_nc(b_core):
    n_chunks = b_core // CHUNK
    nc = bacc.Bacc(None, target_bir_lowering=False, num_swdge_queues=4)
    with tile.TileContext(nc) as tc:
        with tc.tile_pool(name="dram", bufs=1, space="DRAM") as dram:
            word_tab = dram.tile([V + 1, 128], dt.bfloat16, kind="ExternalInput",
                                 name="word_tab", uniquify=False)
            widx_d = dram.tile([P, T * n_chunks * 32], dt.int16, kind="ExternalInput",
                               name="widx", uniquify=False)
            vidx_d = dram.tile([P, n_chunks * T * CHUNK], dt.float16,
                               kind="ExternalInput", name="vidx", uniquify=False)
            tabt_d = dram.tile([P, 14 * 64], dt.bfloat16, kind="ExternalInput",
                               name="tabT", uniquify=False)
            iota_d = dram.tile([P, 1], dt.float32, kind="ExternalInput",
                               name="iota64", uniquify=False)
            ww_d = dram.tile([P, T * H], dt.bfloat16, kind="ExternalInput",
                             name="w_word", uniquify=False)
            wpd_d = dram.tile([P, 2 * T * H], dt.bfloat16, kind="ExternalInput",
                              name="w_pd", uniquify=False)
            wo_d = dram.tile([P, 6 * 96], dt.bfloat16, kind="ExternalInput",
                             name="w_o", uniquify=False)
            bias_d = dram.tile([1, H], dt.bfloat16, kind="ExternalInput",
                               name="bias_row", uniquify=False)
            bo_d = dram.tile([P, 1], dt.float32, kind="ExternalInput",
                             name="bo_pad", uniquify=False)
            out_d = dram.tile([n_chunks * 4, P, OUT], dt.float32, kind="ExternalOutput",
                              name="out", uniquify=False)

            with (
                tc.tile_pool(name="const", bufs=1) as const,
                tc.tile_pool(name="wg", bufs=2) as wg_pool,
                tc.tile_pool(name="vx", bufs=2) as vx_pool,
                tc.tile_pool(name="oh", bufs=2) as oh_pool,
                tc.tile_pool(name="sq", bufs=2) as sq_pool,
                tc.tile_pool(name="h3", bufs=2) as h3_pool,
                tc.tile_pool(name="lg", bufs=2) as lg_pool,
                tc.tile_pool(name="ex", bufs=2) as ex_pool,
                tc.tile_pool(name="sm", bufs=2) as sm_pool,
                tc.tile_pool(name="op", bufs=2) as op_pool,
                tc.tile_pool(name="hps", bufs=1, space="PSUM") as hps_pool,
                tc.tile_pool(name="ltps", bufs=2, space="PSUM") as ltps_pool,
            ):
                ww_sb = const.tile([P, T * H], dt.bfloat16, name="ww_sb")
                nc.sync.dma_start(out=ww_sb[:], in_=ww_d[:])
                wpd_sb = const.tile([P, 2 * T * H], dt.bfloat16, name="wpd_sb")
                nc.sync.dma_start(out=wpd_sb[:], in_=wpd_d[:])
                wo_sb = const.tile([P, 6 * 96], dt.bfloat16, name="wo_sb")
                nc.sync.dma_start(out=wo_sb[:], in_=wo_d[:])
                widx_sb = const.tile([P, T * n_chunks * 32], dt.int16, name="widx_sb")
                nc.sync.dma_start(out=widx_sb[:], in_=widx_d[:])
                tabt_sb = const.tile([P, 14 * 64], dt.bfloat16, name="tabt_sb")
                nc.sync.dma_start(out=tabt_sb[:], in_=tabt_d[:])
                iota_sb = const.tile([P, 1], dt.float32, name="iota_sb")
                nc.sync.dma_start(out=iota_sb[:], in_=iota_d[:])
                bo_sb = const.tile([P, 1], dt.float32, name="bo_sb")
                nc.sync.dma_start(out=bo_sb[:], in_=bo_d[:])
                ident = const.tile([P, P], dt.float32, name="ident")
                make_identity(nc, ident)

                # ---- build proj[s*64+cls] = table_s[cls] @ W_s on device ----
                proj_sb = const.tile([P, T * H], dt.bfloat16, name="proj_sb")
                for t in range(T):
                    pp1 = ltps_pool.tile([P, 512], dt.float32, name="pp1", tag="lt")
                    pp2 = ltps_pool.tile([P, 188], dt.float32, name="pp2", tag="lt")
                    for half in range(2):
                        s = t * 2 + half
                        lhsT = tabt_sb[:, s * 64:(s + 1) * 64]
                        nc.tensor.matmul(pp1[64 * half:64 * half + 64, :], lhsT,
                                         wpd_sb[:, s * H:s * H + 512],
                                         start=True, stop=True)
                        nc.tensor.matmul(pp2[64 * half:64 * half + 64, :], lhsT,
                                         wpd_sb[:, s * H + 512:s * H + 700],
                                         start=True, stop=True)
                    nc.scalar.activation(proj_sb[:, t * H:t * H + 512], pp1[:, :],
                                         mybir.ActivationFunctionType.Copy)
                    nc.scalar.activation(proj_sb[:, t * H + 512:t * H + 700], pp2[:, :],
                                         mybir.ActivationFunctionType.Copy)
                # combined bias rides one-hot row 63 of tile t=0
                nc.sync.dma_start(out=proj_sb[63:64, 0:H], in_=bias_d[:, :])

                qn = 0
                for c in range(n_chunks):
                    # ---- word gathers (feature-major), cycled over queues ----
                    wg = []
                    for t in range(T):
                        g = wg_pool.tile([P, CHUNK], dt.bfloat16, name=f"wg{t}")
                        nc.gpsimd.dma_gather(
                            g.rearrange("p (o n) -> p o n", o=1),
                            word_tab[:],
                            widx_sb[:, (t * n_chunks + c) * 32:(t * n_chunks + c + 1) * 32],
                            CHUNK, CHUNK, 128, transpose=True, queue_num=qn % 4,
                        )
                        qn += 1
                        wg.append(g)

                    # ---- pos/dep one-hots from replicated fp16 idx rows ----
                    vx = vx_pool.tile([P, T * CHUNK], dt.float16, name="vx")
                    nc.sync.dma_start(
                        out=vx[:], in_=vidx_d[:, c * T * CHUNK:(c + 1) * T * CHUNK])
                    oh = []
                    for t in range(T):
                        o = oh_pool.tile([P, CHUNK], dt.bfloat16, name=f"oh{t}")
                        nc.vector.tensor_scalar(
                            o[:, :], vx[:, t * CHUNK:(t + 1) * CHUNK],
                            iota_sb[:, :], None, mybir.AluOpType.is_equal)
                        oh.append(o)

                    # ---- h = x @ W (+bias via one-hot row), h3 = h^2 * h ----
                    h3 = []
                    for mi, (m0, msz) in enumerate(MT):
                        hp = hps_pool.tile([P, CHUNK], dt.float32, name=f"hps{mi}")
                        kb = 0
                        for t in range(T):
                            nc.tensor.matmul(
                                hp[:msz, :],
                                ww_sb[:, t * H + m0: t * H + m0 + msz],
                                wg[t][:, :],
                                start=(kb == 0), stop=(kb == 13),
                            )
                            kb += 1
                        for t in range(T):
                            nc.tensor.matmul(
                                hp[:msz, :],
                                proj_sb[:, t * H + m0: t * H + m0 + msz],
                                oh[t][:, :],
                                start=(kb == 0), stop=(kb == 13),
                            )
                            kb += 1
                        sq = sq_pool.tile([P, CHUNK], dt.float32, name="sq")
                        nc.scalar.square(sq[:msz, :], hp[:msz, :])
                        h3t = h3_pool.tile([P, CHUNK], dt.bfloat16, name=f"h3_{mi}")
                        nc.vector.tensor_mul(h3t[:msz, :], sq[:msz, :], hp[:msz, :])
                        h3.append(h3t)

                    # ---- logits = h3 @ Wo (+bo via ACT bias) ----
                    lg_ps = ltps_pool.tile([P, CHUNK], dt.float32, name="lg_ps", tag="lt")
                    for j, (k0, ksz) in enumerate(LKB):
                        nc.tensor.matmul(
                            lg_ps[:96, :],
                            wo_sb[:ksz, j * 96:(j + 1) * 96],
                            h3[j][:ksz, :],
                            start=(j == 0), stop=(j == 5),
                        )
                    lg_sb = lg_pool.tile([P, CHUNK], dt.float32, name="lg_sb")
                    nc.scalar.activation(lg_sb[:96, :], lg_ps[:96, :],
                                         mybir.ActivationFunctionType.Identity,
                                         bias=bo_sb[:96, :])

                    # ---- transpose to batch-major + softmax (no max sub) ----
                    for q in range(4):
                        tp = ltps_pool.tile([P, 96], dt.float32, name="tp", tag="lt")
                        nc.tensor.transpose(
                            tp[:, :96],
                            lg_sb[:96, q * 128:(q + 1) * 128],
                            ident[:96, :96],
                        )
                        ex = ex_pool.tile([P, 96], dt.float32, name="ex")
                        nc.scalar.activation(ex[:, :], tp[:, :],
                                             mybir.ActivationFunctionType.Exp)
                        sm = sm_pool.tile([P, 1], dt.float32, name="sm")
                        nc.vector.tensor_reduce(sm[:, :], ex[:, :OUT],
                                                axis=mybir.AxisListType.X,
                                                op=mybir.AluOpType.add)
                        rc = sm_pool.tile([P, 1], dt.float32, name="rc")
                        nc.vector.reciprocal(rc[:, :], sm[:, :])
                        opt = op_pool.tile([P, OUT], dt.float32, name="opt")
                        nc.vector.tensor_scalar_mul(opt[:, :], ex[:, :OUT], rc[:, :])
                        nc.sync.dma_start(out=out_d[c * 4 + q], in_=opt[:, :])
    nc.compile()
    return nc


def _wrap_idx(idx_tc):
    """[CHUNK] -> [128, 32] wrapped (i -> [i%16, i//16]) + replicated x8."""
    n = idx_tc.shape[0]
    w = idx_tc.reshape(n // 16, 16).T  # [16, n/16]
    return np.tile(w, (8, 1))


def prep_inputs(word_idx, pos_idx, dep_idx, word_table, pos_table, dep_table,
                Ww, bw, Wp, bp, Wd, bd, Wo, bo, b_core):
    """Returns (shared_map, per_core_fn). Host work is layout-only + small."""
    n_chunks = b_core // CHUNK

    wt = np.zeros((V + 1, 128), dtype=bf16)
    wt[:V, :D] = np.asarray(word_table, np.float32).astype(bf16)

    # pos/dep tables transposed: tabT[p, s*64+cls] = table_s[cls, p]
    tabt = np.zeros((P, 14 * 64), dtype=bf16)
    pt = np.asarray(pos_table, np.float32).astype(bf16)
    dtab = np.asarray(dep_table, np.float32).astype(bf16)
    for t in range(T):
        tabt[:D, (2 * t) * 64:(2 * t) * 64 + NPOS] = pt.T
        tabt[:D, (2 * t + 1) * 64:(2 * t + 1) * 64 + NDEP] = dtab.T

    iota64 = (np.arange(P) % 64).astype(np.float32).reshape(P, 1)

    bias_all = (np.asarray(bw, np.float32) + np.asarray(bp, np.float32)
                + np.asarray(bd, np.float32))
    bias_row = bias_all.astype(bf16).reshape(1, H)

    def pack_w(Wmat):
        arr = np.zeros((T, P, H), dtype=bf16)
        Wmat = np.asarray(Wmat, np.float32)
        for t in range(T):
            arr[t, :D, :] = Wmat[D * t:D * (t + 1), :].astype(bf16)
        return arr

    ww = pack_w(Ww)
    wp = pack_w(Wp)
    wd = pack_w(Wd)
    # proj-build rhs: [128, s*700..] with s = 2t (pos) / 2t+1 (dep);
    # partitions 0..99 = feature dim of W rows for slot s.
    wpd = np.zeros((T, 2, P, H), dtype=bf16)
    wpd[:, 0] = wp
    wpd[:, 1] = wd

    wo = np.zeros((6, P, 96), dtype=bf16)
    Wo32 = np.asarray(Wo, np.float32)
    for j, (k0, ksz) in enumerate(LKB):
        wo[j, :ksz, :OUT] = Wo32[k0:k0 + ksz, :].astype(bf16)

    bo_pad = np.zeros((P, 1), dtype=np.float32)
    bo_pad[:OUT, 0] = np.asarray(bo, np.float32)

    shared = {
        "word_tab": wt,
        "tabT": tabt,
        "iota64": iota64,
        "bias_row": bias_row,
        "w_word": np.ascontiguousarray(ww.transpose(1, 0, 2)).reshape(P, T * H),
        "w_pd": np.ascontiguousarray(wpd.transpose(2, 0, 1, 3)).reshape(P, 2 * T * H),
        "w_o": np.ascontiguousarray(wo.transpose(1, 0, 2)).reshape(P, 6 * 96),
        "bo_pad": bo_pad,
    }

    wi = np.asarray(word_idx, np.int64).copy()
    wi[wi < 0] = V
    wi = wi.astype(np.int16)
    pi16 = np.asarray(pos_idx, np.int32).astype(np.float16)
    di16 = np.asarray(dep_idx, np.int32).astype(np.float16)

    def core_map(core):
        s = slice(core * b_core, (core + 1) * b_core)
        wic = wi[s]
        widx = np.zeros((P, T, n_chunks, 32), dtype=np.int16)
        for t in range(T):
            for c in range(n_chunks):
                widx[:, t, c, :] = _wrap_idx(wic[c * CHUNK:(c + 1) * CHUNK, t])

        # vidx[p, c, t, i]: p<64 -> pos_idx, p>=64 -> dep_idx; (t=0, p=63) = 63
        pc = pi16[s].reshape(n_chunks, CHUNK, T).transpose(0, 2, 1)  # [nch,T,512]
        dc = di16[s].reshape(n_chunks, CHUNK, T).transpose(0, 2, 1)
        vidx = np.empty((P, n_chunks, T, CHUNK), dtype=np.float16)
        vidx[:64] = pc[None, :, :, :]
        vidx[64:] = dc[None, :, :, :]
        vidx[63, :, 0, :] = np.float16(63.0)

        m = dict(shared)
        m["widx"] = widx.reshape(P, T * n_chunks * 32)
        m["vidx"] = np.ascontiguousarray(vidx).reshape(P, n_chunks * T * CHUNK)
        return m

    return shared, core_map


def kernel(**inputs):
    b_core = B_CORE
    if b_core not in _NC_CACHE:
        _NC_CACHE[b_core] = build_nc(b_core)
    nc = _NC_CACHE[b_core]

    _, core_map = prep_inputs(b_core=b_core, **inputs)
    in_maps = [core_map(i) for i in range(NCORES)]
    res = run_bass_kernel_spmd(nc, in_maps, core_ids=list(range(NCORES)))
    outs = [r["out"].reshape(b_core, OUT) for r in res.results]
    return np.concatenate(outs, axis=0).astype(np.float32)


# revision 15
# speedup vs baseline: 2.2203x; 1.0260x over previous
"""Trainium2 Bass kernel for nn_DependencyParsing (embedding_lookup).

Strategy (pure data-parallel over 8 NeuronCores, B=65536 -> 8192/core):
  - word_table cast to bf16, rows padded to 256B; word embeddings gathered
    feature-major straight into SBUF via SWDGE transpose dma_gather
    (512 idx / instruction HW cap), cycled over 2 SWDGE queues (one
    queue's descriptor ring serializes at ~4.75us/gather; 3+ queues
    corrupt gather packets when HWDGE traffic runs concurrently).
  - pos/dep lookups use no gather at all: pe@Wp + de@Wd is computed as
    onehot @ proj, where proj[s*64+cls] = table_s[cls] @ W_s (built on
    device, 28 small matmuls) and the one-hot [128, 512] per (t) comes
    from a single DVE is_equal of host-replicated fp16 index rows against
    a per-partition iota. Slot s=pos_t on partitions 0..63, dep_t on
    64..127. The combined bias (bw+bp+bd) rides a constant-1 row (t=0,
    partition 63) with proj row 63 = bias.
  - h = x @ W as bf16 matmuls (14 K-blocks x 6 M-tiles per 512 chunk)
    accumulating f32 in PSUM; h^3 = Square(h)*h on ACT+DVE -> bf16.
  - logits = h3 @ Wo (bo added via ACT Identity-with-bias on the PSUM
    eviction), PE-mode transpose to batch-major, then a max-free softmax
    (logits are tiny so exp is safe) on ACT/DVE.
"""

import os

import numpy as np
import ml_dtypes

import concourse.bacc as bacc
import concourse.mybir as mybir
import concourse.tile as tile
from concourse.masks import make_identity
from concourse.bass_utils import run_bass_kernel_spmd

B, T, D, H, V, NPOS, NDEP, OUT = 65536, 7, 100, 700, 32000, 50, 45, 93
NCORES = 8
B_CORE = B // NCORES
CHUNK = 512
P = 128
# M-tiles over the 700 output features of h
MT = [(0, 128), (128, 128), (256, 128), (384, 128), (512, 128), (640, 60)]
# K-blocks for logits: 700 h-features in 6 blocks of 128 (last 60)
LKB = [(0, 128), (128, 128), (256, 128), (384, 128), (512, 128), (640, 60)]
dt = mybir.dt
bf16 = ml_dtypes.bfloat16
NQ = int(os.environ.get("KERNEL_NQ", "2"))

_NC_CACHE = {}


def build_nc(b_core):
    n_chunks = b_core // CHUNK
    nc = bacc.Bacc(None, target_bir_lowering=False, num_swdge_queues=4)
    with tile.TileContext(nc) as tc:
        with tc.tile_pool(name="dram", bufs=1, space="DRAM") as dram:
            word_tab = dram.tile([V + 1, 128], dt.bfloat16, kind="ExternalInput",
                                 name="word_tab", uniquify=False)
            widx_d = dram.tile([P, T * n_chunks * 32], dt.int16, kind="ExternalInput",
                               name="widx", uniquify=False)
            vidx_d = dram.tile([P, n_chunks * T * CHUNK], dt.float16,
                               kind="ExternalInput", name="vidx", uniquify=False)
            tabt_d = dram.tile([P, 14 * 64], dt.bfloat16, kind="ExternalInput",
                               name="tabT", uniquify=False)
            iota_d = dram.tile([P, 1], dt.float32, kind="ExternalInput",
                               name="iota64", uniquify=False)
            ww_d = dram.tile([P, T * H], dt.bfloat16, kind="ExternalInput",
                             name="w_word", uniquify=False)
            wpd_d = dram.tile([P, 2 * T * H], dt.bfloat16, kind="ExternalInput",
                              name="w_pd", uniquify=False)
            wo_d = dram.tile([P, 6 * 96], dt.bfloat16, kind="ExternalInput",
                             name="w_o", uniquify=False)
            bias_d = dram.tile([1, H], dt.bfloat16, kind="ExternalInput",
                               name="bias_row", uniquify=False)
            bo_d = dram.tile([P, 1], dt.float32, kind="ExternalInput",
                             name="bo_pad", uniquify=False)
            out_d = dram.tile([n_chunks * 4, P, OUT], dt.float32, kind="ExternalOutput",
                              name="out", uniquify=False)

            with (
                tc.tile_pool(name="const", bufs=1) as const,
                tc.tile_pool(name="wg", bufs=2) as wg_pool,
                tc.tile_pool(name="vx", bufs=2) as vx_pool,
                tc.tile_pool(name="oh", bufs=2) as oh_pool,
                tc.tile_pool(name="sq", bufs=2) as sq_pool,
                tc.tile_pool(name="h3", bufs=2) as h3_pool,
                tc.tile_pool(name="lg", bufs=2) as lg_pool,
                tc.tile_pool(name="ex", bufs=2) as ex_pool,
                tc.tile_pool(name="sm", bufs=2) as sm_pool,
                tc.tile_pool(name="op", bufs=2) as op_pool,
                tc.tile_pool(name="hps", bufs=1, space="PSUM") as hps_pool,
                tc.tile_pool(name="ltps", bufs=2, space="PSUM") as ltps_pool,
            ):
                ww_sb = const.tile([P, T * H], dt.bfloat16, name="ww_sb")
                nc.sync.dma_start(out=ww_sb[:], in_=ww_d[:])
                wpd_sb = const.tile([P, 2 * T * H], dt.bfloat16, name="wpd_sb")
                nc.sync.dma_start(out=wpd_sb[:], in_=wpd_d[:])
                wo_sb = const.tile([P, 6 * 96], dt.bfloat16, name="wo_sb")
                nc.sync.dma_start(out=wo_sb[:], in_=wo_d[:])
                widx_sb = const.tile([P, T * n_chunks * 32], dt.int16, name="widx_sb")
                nc.sync.dma_start(out=widx_sb[:], in_=widx_d[:])
                tabt_sb = const.tile([P, 14 * 64], dt.bfloat16, name="tabt_sb")
                nc.sync.dma_start(out=tabt_sb[:], in_=tabt_d[:])
                iota_sb = const.tile([P, 1], dt.float32, name="iota_sb")
                nc.sync.dma_start(out=iota_sb[:], in_=iota_d[:])
                bo_sb = const.tile([P, 1], dt.float32, name="bo_sb")
                nc.sync.dma_start(out=bo_sb[:], in_=bo_d[:])
                ident = const.tile([P, P], dt.float32, name="ident")
                make_identity(nc, ident)

                # ---- build proj[s*64+cls] = table_s[cls] @ W_s on device ----
                proj_sb = const.tile([P, T * H], dt.bfloat16, name="proj_sb")
                for t in range(T):
                    pp1 = ltps_pool.tile([P, 512], dt.float32, name="pp1", tag="lt")
                    pp2 = ltps_pool.tile([P, 188], dt.float32, name="pp2", tag="lt")
                    for half in range(2):
                        s = t * 2 + half
                        lhsT = tabt_sb[:, s * 64:(s + 1) * 64]
                        nc.tensor.matmul(pp1[64 * half:64 * half + 64, :], lhsT,
                                         wpd_sb[:, s * H:s * H + 512],
                                         start=True, stop=True)
                        nc.tensor.matmul(pp2[64 * half:64 * half + 64, :], lhsT,
                                         wpd_sb[:, s * H + 512:s * H + 700],
                                         start=True, stop=True)
                    nc.scalar.activation(proj_sb[:, t * H:t * H + 512], pp1[:, :],
                                         mybir.ActivationFunctionType.Copy)
                    nc.scalar.activation(proj_sb[:, t * H + 512:t * H + 700], pp2[:, :],
                                         mybir.ActivationFunctionType.Copy)
                # combined bias rides one-hot row 63 of tile t=0
                nc.sync.dma_start(out=proj_sb[63:64, 0:H], in_=bias_d[:, :])

                qn = 0
                for c in range(n_chunks):
                    # ---- word gathers (feature-major), cycled over queues ----
                    wg = []
                    for t in range(T):
                        g = wg_pool.tile([P, CHUNK], dt.bfloat16, name=f"wg{t}")
                        nc.gpsimd.dma_gather(
                            g.rearrange("p (o n) -> p o n", o=1),
                            word_tab[:],
                            widx_sb[:, (t * n_chunks + c) * 32:(t * n_chunks + c + 1) * 32],
                            CHUNK, CHUNK, 128, transpose=True, queue_num=qn % NQ,
                        )
                        qn += 1
                        wg.append(g)

                    # ---- pos/dep one-hots from replicated fp16 idx rows ----
                    vx = vx_pool.tile([P, T * CHUNK], dt.float16, name="vx")
                    nc.sync.dma_start(
                        out=vx[:], in_=vidx_d[:, c * T * CHUNK:(c + 1) * T * CHUNK])
                    oh = []
                    for t in range(T):
                        o = oh_pool.tile([P, CHUNK], dt.bfloat16, name=f"oh{t}")
                        nc.vector.tensor_scalar(
                            o[:, :], vx[:, t * CHUNK:(t + 1) * CHUNK],
                            iota_sb[:, :], None, mybir.AluOpType.is_equal)
                        oh.append(o)

                    # ---- h = x @ W (+bias via one-hot row), h3 = h^2 * h ----
                    h3 = []
                    for mi, (m0, msz) in enumerate(MT):
                        hp = hps_pool.tile([P, CHUNK], dt.float32, name=f"hps{mi}")
                        kb = 0
                        for t in range(T):
                            nc.tensor.matmul(
                                hp[:msz, :],
                                ww_sb[:, t * H + m0: t * H + m0 + msz],
                                wg[t][:, :],
                                start=(kb == 0), stop=(kb == 13),
                            )
                            kb += 1
                        for t in range(T):
                            nc.tensor.matmul(
                                hp[:msz, :],
                                proj_sb[:, t * H + m0: t * H + m0 + msz],
                                oh[t][:, :],
                                start=(kb == 0), stop=(kb == 13),
                            )
                            kb += 1
                        sq = sq_pool.tile([P, CHUNK], dt.float32, name="sq")
                        nc.scalar.square(sq[:msz, :], hp[:msz, :])
                        h3t = h3_pool.tile([P, CHUNK], dt.bfloat16, name=f"h3_{mi}")
                        nc.vector.tensor_mul(h3t[:msz, :], sq[:msz, :], hp[:msz, :])
                        h3.append(h3t)

                    # ---- logits = h3 @ Wo (+bo via ACT bias) ----
                    lg_ps = ltps_pool.tile([P, CHUNK], dt.float32, name="lg_ps", tag="lt")
                    for j, (k0, ksz) in enumerate(LKB):
                        nc.tensor.matmul(
                            lg_ps[:96, :],
                            wo_sb[:ksz, j * 96:(j + 1) * 96],
                            h3[j][:ksz, :],
                            start=(j == 0), stop=(j == 5),
                        )
                    lg_sb = lg_pool.tile([P, CHUNK], dt.float32, name="lg_sb")
                    nc.scalar.activation(lg_sb[:96, :], lg_ps[:96, :],
                                         mybir.ActivationFunctionType.Identity,
                                         bias=bo_sb[:96, :])

                    # ---- transpose to batch-major + softmax (no max sub) ----
                    for q in range(4):
                        tp = ltps_pool.tile([P, 96], dt.float32, name="tp", tag="lt")
                        nc.tensor.transpose(
                            tp[:, :96],
                            lg_sb[:96, q * 128:(q + 1) * 128],
                            ident[:96, :96],
                        )
                        ex = ex_pool.tile([P, 96], dt.float32, name="ex")
                        nc.scalar.activation(ex[:, :], tp[:, :],
                                             mybir.ActivationFunctionType.Exp)
                        sm = sm_pool.tile([P, 1], dt.float32, name="sm")
                        nc.vector.tensor_reduce(sm[:, :], ex[:, :OUT],
                                                axis=mybir.AxisListType.X,
                                                op=mybir.AluOpType.add)
                        rc = sm_pool.tile([P, 1], dt.float32, name="rc")
                        nc.vector.reciprocal(rc[:, :], sm[:, :])
                        opt = op_pool.tile([P, OUT], dt.float32, name="opt")
                        nc.vector.tensor_scalar_mul(opt[:, :], ex[:, :OUT], rc[:, :])
                        nc.sync.dma_start(out=out_d[c * 4 + q], in_=opt[:, :])
    nc.compile()
    return nc


def _wrap_idx(idx_tc):
    """[CHUNK] -> [128, 32] wrapped (i -> [i%16, i//16]) + replicated x8."""
    n = idx_tc.shape[0]
    w = idx_tc.reshape(n // 16, 16).T  # [16, n/16]
    return np.tile(w, (8, 1))


def prep_inputs(word_idx, pos_idx, dep_idx, word_table, pos_table, dep_table,
                Ww, bw, Wp, bp, Wd, bd, Wo, bo, b_core):
    """Returns (shared_map, per_core_fn). Host work is layout-only + small."""
    n_chunks = b_core // CHUNK

    wt = np.zeros((V + 1, 128), dtype=bf16)
    wt[:V, :D] = np.asarray(word_table, np.float32).astype(bf16)

    # pos/dep tables transposed: tabT[p, s*64+cls] = table_s[cls, p]
    tabt = np.zeros((P, 14 * 64), dtype=bf16)
    pt = np.asarray(pos_table, np.float32).astype(bf16)
    dtab = np.asarray(dep_table, np.float32).astype(bf16)
    for t in range(T):
        tabt[:D, (2 * t) * 64:(2 * t) * 64 + NPOS] = pt.T
        tabt[:D, (2 * t + 1) * 64:(2 * t + 1) * 64 + NDEP] = dtab.T

    iota64 = (np.arange(P) % 64).astype(np.float32).reshape(P, 1)

    bias_all = (np.asarray(bw, np.float32) + np.asarray(bp, np.float32)
                + np.asarray(bd, np.float32))
    bias_row = bias_all.astype(bf16).reshape(1, H)

    def pack_w(Wmat):
        arr = np.zeros((T, P, H), dtype=bf16)
        Wmat = np.asarray(Wmat, np.float32)
        for t in range(T):
            arr[t, :D, :] = Wmat[D * t:D * (t + 1), :].astype(bf16)
        return arr

    ww = pack_w(Ww)
    wp = pack_w(Wp)
    wd = pack_w(Wd)
    # proj-build rhs: [128, s*700..] with s = 2t (pos) / 2t+1 (dep);
    # partitions 0..99 = feature dim of W rows for slot s.
    wpd = np.zeros((T, 2, P, H), dtype=bf16)
    wpd[:, 0] = wp
    wpd[:, 1] = wd

    wo = np.zeros((6, P, 96), dtype=bf16)
    Wo32 = np.asarray(Wo, np.float32)
    for j, (k0, ksz) in enumerate(LKB):
        wo[j, :ksz, :OUT] = Wo32[k0:k0 + ksz, :].astype(bf16)

    bo_pad = np.zeros((P, 1), dtype=np.float32)
    bo_pad[:OUT, 0] = np.asarray(bo, np.float32)

    shared = {
        "word_tab": wt,
        "tabT": tabt,
        "iota64": iota64,
        "bias_row": bias_row,
        "w_word": np.ascontiguousarray(ww.transpose(1, 0, 2)).reshape(P, T * H),
        "w_pd": np.ascontiguousarray(wpd.transpose(2, 0, 1, 3)).reshape(P, 2 * T * H),
        "w_o": np.ascontiguousarray(wo.transpose(1, 0, 2)).reshape(P, 6 * 96),
        "bo_pad": bo_pad,
    }

    wi = np.asarray(word_idx, np.int64).copy()
    wi[wi < 0] = V
    wi = wi.astype(np.int16)
    pi16 = np.asarray(pos_idx, np.int32).astype(np.float16)
    di16 = np.asarray(dep_idx, np.int32).astype(np.float16)

    def core_map(core):
        s = slice(core * b_core, (core + 1) * b_core)
        wic = wi[s]
        widx = np.zeros((P, T, n_chunks, 32), dtype=np.int16)
        for t in range(T):
            for c in range(n_chunks):
                widx[:, t, c, :] = _wrap_idx(wic[c * CHUNK:(c + 1) * CHUNK, t])

        # vidx[p, c, t, i]: p<64 -> pos_idx, p>=64 -> dep_idx; (t=0, p=63) = 63
        pc = pi16[s].reshape(n_chunks, CHUNK, T).transpose(0, 2, 1)  # [nch,T,512]
        dc = di16[s].reshape(n_chunks, CHUNK, T).transpose(0, 2, 1)
        vidx = np.empty((P, n_chunks, T, CHUNK), dtype=np.float16)
        vidx[:64] = pc[None, :, :, :]
        vidx[64:] = dc[None, :, :, :]
        vidx[63, :, 0, :] = np.float16(63.0)

        m = dict(shared)
        m["widx"] = widx.reshape(P, T * n_chunks * 32)
        m["vidx"] = np.ascontiguousarray(vidx).reshape(P, n_chunks * T * CHUNK)
        return m

    return shared, core_map


def kernel(**inputs):
    b_core = B_CORE
    if b_core not in _NC_CACHE:
        _NC_CACHE[b_core] = build_nc(b_core)
    nc = _NC_CACHE[b_core]

    _, core_map = prep_inputs(b_core=b_core, **inputs)
    in_maps = [core_map(i) for i in range(NCORES)]
    res = run_bass_kernel_spmd(nc, in_maps, core_ids=list(range(NCORES)))
    outs = [r["out"].reshape(b_core, OUT) for r in res.results]
    return np.concatenate(outs, axis=0).astype(np.float32)
